# revision 32
# baseline (speedup 1.0000x reference)
"""Causal self-attention Bass/Tile kernel for 8 Trainium2 NeuronCores.

Problem (hardcoded): x (4, 2048, 1024) f32, w_attn (1024, 3072), w_proj
(1024, 1024).  H=16 heads, D=64.  Output: (4, 2048, 1024) f32.

Sharding: core c handles batch b = c // 2 and head-group hg = c % 2
(8 heads each).  Data parallel on B, tensor parallel on heads: each core
gets the w_attn columns for its heads (q|k|v, each 512 cols) and the
w_proj rows for its heads (512 rows).  Per-core output is a partial sum
over head groups; the host adds the two partials per batch.

Per-core kernel structure (strips of 512 queries):
  phase 1: PE-transpose x strip -> x^T (exact f32); matmuls produce
           Q^T/K^T ([d, tok], head pairs stacked on partitions) and V
           ([tok, d], 8 heads side by side).
  phase 2: per head-pair, per key-tile t: scores^T = K^T.T @ Q^T
           (row-packed pair), exp on ACT (scale=1/8 folded in), causal
           mask multiply on diagonal tiles (DVE), then col-packed
           matmuls accumulate exp@V -> y^T and ones@exp -> row sums.
           Softmax normalization = DVE reciprocal of sums + multiply.
  phase 3: out = y^T.T @ w_proj, accumulated over the 4 local f-chunks
           (emitted one strip late so the PE has transpose work while
           DVE normalizes).

Matmul dtype is configurable per phase: float32 (exact, 4 cyc/row) or
float32r (fp32 with 11-bit mantissa, 1 cyc/row).  float32r operands must
be *produced* rounded: on-chip producers (DVE copies, ACT exp) write
f32r-typed tiles, and weights are pre-rounded on the host (the DRAM
tensors are declared f32r).

No softmax max-subtraction: scores for these inputs are ~N(0,1)
(measured |s| <= 8.4), exp is fp32-safe.

PSUM static budget (8 banks): ph1 shared tag x4 (transpose/qkv/proj),
ps x2 (scores), py x1 (exp@V accum), psm x1 (softmax sums accum).
"""

import os
from contextlib import ExitStack

import numpy as np

import concourse.bass as bass
import concourse.bacc as bacc
import concourse.mybir as mybir
import concourse.tile as tile
from concourse.bass_utils import run_bass_kernel_spmd
from concourse.masks import make_identity

F32 = mybir.dt.float32
F32R = mybir.dt.float32r
EXP = mybir.ActivationFunctionType.Exp

S = 2048          # sequence length
E = 1024          # embedding
D = 64            # head dim
HL = 8            # heads per core
NP = 4            # head pairs per core
EC = 8            # E / 128 chunks
NSTRIP = 4        # query strips of 512
TPS = 4           # 128-token tiles per strip
NT = 16           # 128-key tiles total

_DT = {"f32": F32, "f32r": F32R}
MM_QKV = _DT[os.environ.get("MM_QKV", "f32r")]
MM_ATT = _DT[os.environ.get("MM_ATT", "f32r")]
MM_PROJ = _DT[os.environ.get("MM_PROJ", "f32r")]


def emit_kernel(ctx, tc, out, x, w_qkv, w_proj):
    nc = tc.nc

    const = ctx.enter_context(tc.tile_pool(name="const", bufs=1))
    wpool = ctx.enter_context(tc.tile_pool(name="weights", bufs=1))
    kv = ctx.enter_context(tc.tile_pool(name="kv", bufs=1))
    work = ctx.enter_context(tc.tile_pool(name="work", bufs=1))
    psum = ctx.enter_context(tc.tile_pool(name="psum", bufs=1, space="PSUM"))

    # ---- constants ----
    ident = const.tile([128, 128], F32, name="ident")
    make_identity(nc, ident)
    # ones column source for the V||1 augmented tiles (f32; rounded on copy)
    ones_row8 = const.tile([128, 8], F32, name="ones_row8")
    nc.gpsimd.memset(ones_row8[:], 1.0)
    # DRAM bounce rows for the softmax-sums broadcast (2 per pair-strip)
    rbounce = nc.dram_tensor("rbounce", [2 * NP * NSTRIP, 512], F32).ap()
    # causal mask template: T[i, jj] = 1 iff jj >= i + 384.
    # mask variant p (key tile 4s+p vs strip s) = T[:, 384-128p : 896-128p]
    mask_t = const.tile([128, 896], F32, name="mask_t")
    nc.gpsimd.memset(mask_t[:], 1.0)
    nc.gpsimd.affine_select(
        out=mask_t[:],
        in_=mask_t[:],
        compare_op=mybir.AluOpType.is_ge,
        fill=0.0,
        base=-384,
        channel_multiplier=-1,
        pattern=[[1, 896]],
    )

    # ---- resident weights (DRAM already in matmul dtype, host-rounded) ----
    wqk = []
    for e in range(EC):
        t = wpool.tile([128, 1024], MM_QKV, name=f"wqk{e}", tag=f"wqk{e}")
        nc.sync.dma_start(out=t[:], in_=w_qkv[e * 128:(e + 1) * 128, 0:1024])
        wqk.append(t)
    wv = []
    for e in range(EC):
        t = wpool.tile([128, 512], MM_QKV, name=f"wv{e}", tag=f"wv{e}")
        nc.sync.dma_start(out=t[:], in_=w_qkv[e * 128:(e + 1) * 128, 1024:1536])
        wv.append(t)
    wpj = []
    for f in range(NP):
        t = wpool.tile([128, 1024], MM_PROJ, name=f"wpj{f}", tag=f"wpj{f}")
        nc.sync.dma_start(out=t[:], in_=w_proj[f * 128:(f + 1) * 128, :])
        wpj.append(t)

    # ---- persistent K^T (pair-stacked) and V||ones (8 heads x 65) ----
    kT = [kv.tile([128, S], MM_ATT, name=f"kT{p}", tag=f"kT{p}")
          for p in range(NP)]
    vaug = [kv.tile([128, 520], MM_ATT, name=f"vaug_{t}", tag=f"vaug_{t}")
            for t in range(NT)]

    state = {}

    def transpose_chunk(s, tts):
        """Load + PE-transpose x tiles `tts` of strip s into x^T."""
        if ("xT", s) not in state:
            state[("xT", s)] = [
                work.tile([128, 512], MM_QKV, name=f"xT{e}_{s}", tag=f"xT{e}")
                for e in range(EC)]
        xT = state[("xT", s)]
        for tt in tts:
            for half in range(2):
                xin = work.tile([128, 512], F32, name=f"xin_{s}_{tt}_{half}",
                                tag="xin", bufs=2)
                r0 = (s * TPS + tt) * 128
                nc.sync.dma_start(
                    out=xin[:],
                    in_=x[r0:r0 + 128, half * 512:(half + 1) * 512])
                for e4 in range(4):
                    e = half * 4 + e4
                    pt = psum.tile([128, 128], F32, name=f"pt_{s}_{tt}_{e}",
                                   tag="ph1", bufs=3)
                    nc.tensor.transpose(pt[:], xin[:, e4 * 128:(e4 + 1) * 128],
                                        ident[:])
                    nc.vector.tensor_copy(xT[e][:, tt * 128:(tt + 1) * 128],
                                          pt[:])

    def qk_chunk(s, ps):
        """Q^T/K^T for pairs `ps` of strip s."""
        xT = state[("xT", s)]
        if ("qT", s) not in state:
            state[("qT", s)] = [
                work.tile([128, 512], MM_ATT, name=f"qT{p}_{s}",
                          tag=f"qT{p}", bufs=2)
                for p in range(NP)]
        qT = state[("qT", s)]
        for p in ps:
            pq = psum.tile([128, 512], F32, name=f"pq_{s}_{p}", tag="ph1",
                           bufs=3)
            pk = psum.tile([128, 512], F32, name=f"pk_{s}_{p}", tag="ph1",
                           bufs=3)
            for e in range(EC):
                nc.tensor.matmul(
                    pq[:], wqk[e][:, p * 128:(p + 1) * 128],
                    xT[e][:], start=(e == 0), stop=(e == EC - 1))
            for e in range(EC):
                nc.tensor.matmul(
                    pk[:], wqk[e][:, 512 + p * 128:512 + (p + 1) * 128],
                    xT[e][:], start=(e == 0), stop=(e == EC - 1))
            nc.vector.tensor_copy(qT[p][:], pq[:])
            nc.vector.tensor_copy(kT[p][:, s * 512:(s + 1) * 512], pk[:])

    def v_chunk(s, tts):
        """V||ones for x tiles `tts` of strip s."""
        xT = state[("xT", s)]
        for tt in tts:
            pv = psum.tile([128, 512], F32, name=f"pv_{s}_{tt}", tag="ph1",
                           bufs=3)
            for e in range(EC):
                nc.tensor.matmul(
                    pv[:], xT[e][:, tt * 128:(tt + 1) * 128],
                    wv[e][:], start=(e == 0), stop=(e == EC - 1))
            # scatter V into the augmented [head*65 .. head*65+64] slots and
            # fill the ones columns, both as single strided copies
            va = vaug[s * TPS + tt]
            va3 = va.rearrange("p (h c) -> p h c", c=65)
            nc.vector.tensor_copy(va3[:, :, 0:64],
                                  pv[:].rearrange("p (h c) -> p h c", c=64))
            nc.vector.tensor_copy(va3[:, :, 64:65],
                                  ones_row8[:].rearrange("p (h c) -> p h c", c=1))

    def phase1_chunks(s):
        """Phase-1 work for strip s as schedulable chunks."""
        return [
            lambda: transpose_chunk(s, (0, 1)),
            lambda: transpose_chunk(s, (2, 3)),
            lambda: qk_chunk(s, (0, 1)),
            lambda: (qk_chunk(s, (2, 3)), v_chunk(s, (0, 1, 2, 3))),
        ]

    def phase2(s, fillers):
        """Attention for strip s; `fillers` are emission closures injected
        between the per-pair normalize stages so the PE/DVE have independent
        work while ACT paces attention and the softmax DMA bounce lands.
        Returns normalized yT tiles."""
        qT = state[("qT", s)]
        yT = [work.tile([128, 512], MM_PROJ, name=f"yT{p}_{s}", tag=f"yT{p}")
              for p in range(NP)]
        ntile = 4 * s + 4
        fillers = list(fillers)
        for p in range(NP):
            py_a = psum.tile([65, 512], F32, name=f"pya_{s}_{p}", tag="py",
                             bufs=2)
            py_b = psum.tile([65, 512], F32, name=f"pyb_{s}_{p}", tag="py",
                             bufs=2)

            def scores_exp(t):
                # diagonal tiles: columns below 128*dshift are fully masked,
                # so compute only [c0:512] (c0 capped at 256 to keep the
                # f32r matmul in its fast >=256-free-dim regime)
                dshift = t - 4 * s
                c0 = 0 if dshift < 0 else min(128 * dshift, 256)
                ksl = kT[p][:, t * 128:(t + 1) * 128]
                ps_a = psum.tile([128, 512], F32, name=f"psa_{s}_{p}_{t}",
                                 tag="ps", bufs=3)
                ps_b = psum.tile([128, 512], F32, name=f"psb_{s}_{p}_{t}",
                                 tag="ps", bufs=3)
                nc.tensor.matmul(ps_a[:, c0:], ksl[0:64, :], qT[p][0:64, c0:],
                                 start=True, stop=True)
                nc.tensor.matmul(ps_b[:, c0:], ksl[64:128, :],
                                 qT[p][64:128, c0:],
                                 start=True, stop=True,
                                 tile_position=(64, 0))
                es_a = work.tile([128, 512], MM_ATT, name=f"esa_{s}_{p}_{t}",
                                 tag="es", bufs=4)
                es_b = work.tile([128, 512], MM_ATT, name=f"esb_{s}_{p}_{t}",
                                 tag="es", bufs=4)
                nc.scalar.activation(es_a[:, c0:], ps_a[:, c0:], EXP,
                                     scale=0.125)
                nc.scalar.activation(es_b[:, c0:], ps_b[:, c0:], EXP,
                                     scale=0.125)
                if dshift >= 0:  # causal mask on the partially-valid span
                    if dshift == 3:
                        sl, base, w = slice(256, 512), -128, 256
                    else:
                        sl = slice(128 * dshift, 128 * dshift + 128)
                        base, w = 0, 128
                    for est in (es_a, es_b):
                        nc.gpsimd.affine_select(
                            out=est[:, sl], in_=est[:, sl],
                            compare_op=mybir.AluOpType.is_ge, fill=0.0,
                            base=base, channel_multiplier=-1,
                            pattern=[[1, w]])
                return es_a, es_b, c0

            def av_sums(t, es_a, es_b, c0):
                st = (t == 0)
                sp = (t == ntile - 1)
                vA = vaug[t][:, (2 * p) * 65:(2 * p) * 65 + 65]
                vB = vaug[t][:, (2 * p + 1) * 65:(2 * p + 1) * 65 + 65]
                nc.tensor.matmul(py_a[:, c0:], vA, es_a[:, c0:],
                                 start=st, stop=sp)
                nc.tensor.matmul(py_b[:, c0:], vB, es_b[:, c0:],
                                 start=st, stop=sp)

            # software pipeline: issue scores(t+1) before exp@V(t) so the
            # PE never waits on ACT's exp.
            prev = scores_exp(0)
            for t in range(1, ntile):
                cur = scores_exp(t)
                av_sums(t - 1, *prev)
                prev = cur
            av_sums(ntile - 1, *prev)
            del prev

            # softmax normalization: bounce the two sums rows (row 64 of
            # py_*) through DRAM to broadcast them over 64 partitions each,
            # one in-place reciprocal, two DVE multiplies (head B
            # partition-shifted into yT rows 64:128).  A filler chunk is
            # emitted between the bounce-out and the reciprocal so the DMA
            # round-trip doesn't stall the DVE pipeline.
            ri = 2 * (s * NP + p)
            srab = work.tile([1, 1024], F32, name=f"srab_{s}_{p}",
                             tag="srab", bufs=2)
            nc.vector.tensor_copy(srab[:, 0:512], py_a[64:65, :])
            nc.vector.tensor_copy(srab[:, 512:1024], py_b[64:65, :])
            nc.scalar.dma_start(
                out=rbounce[ri:ri + 2, :].rearrange("a b -> (a b)").unsqueeze(0),
                in_=srab[:])
            recb = work.tile([128, 512], F32, name=f"recb_{s}_{p}",
                             tag="recb", bufs=2)
            nc.scalar.dma_start(
                out=recb[0:64, :],
                in_=rbounce[ri:ri + 1, :].broadcast_to((64, 512)))
            nc.scalar.dma_start(
                out=recb[64:128, :],
                in_=rbounce[ri + 1:ri + 2, :].broadcast_to((64, 512)))
            if fillers:
                fillers.pop(0)()
            nc.vector.reciprocal(recb[:], recb[:])
            nc.vector.tensor_mul(yT[p][0:64, :], py_a[0:64, :], recb[0:64, :])
            nc.vector.tensor_mul(yT[p][64:128, :], py_b[0:64, :],
                                 recb[64:128, :])
        for f in fillers:
            f()
        return yT

    def phase3(s, yT):
        """Projection for strip s: out partial = y^T.T @ w_proj."""
        for tt in range(TPS):
            for eo in range(2):
                po = psum.tile([128, 512], F32, name=f"po_{s}_{tt}_{eo}",
                               tag="ph1", bufs=3)
                for p in range(NP):
                    nc.tensor.matmul(
                        po[:], yT[p][:, tt * 128:(tt + 1) * 128],
                        wpj[p][:, eo * 512:(eo + 1) * 512],
                        start=(p == 0), stop=(p == NP - 1))
                osb = work.tile([128, 512], F32, name=f"osb_{s}_{tt}_{eo}",
                                tag="osb", bufs=2)
                nc.vector.tensor_copy(osb[:], po[:])
                r0 = (s * TPS + tt) * 128
                nc.sync.dma_start(
                    out=out[r0:r0 + 128, eo * 512:(eo + 1) * 512], in_=osb[:])

    def whole_body():
        state.clear()
        for c in phase1_chunks(0):
            c()
        prev_yT = None
        for s in range(NSTRIP):
            fillers = []
            if prev_yT is not None:
                yts = prev_yT
                fillers.append(lambda yts=yts, ss=s - 1: phase3(ss, yts))
            if s + 1 < NSTRIP:
                fillers.extend(phase1_chunks(s + 1))
            prev_yT = phase2(s, fillers)
        phase3(NSTRIP - 1, prev_yT)

    repeat = int(os.environ.get("KREPEAT", "1"))
    if repeat > 1:
        # timing-only mode: run the whole computation `repeat` times
        # (idempotent) so marginal wall-clock per iteration = HW exec time
        with tc.For_i(0, repeat, 1):
            whole_body()
    else:
        whole_body()


_CACHE = {}


def build_nc():
    if "nc" in _CACHE:
        return _CACHE["nc"]
    nc = bacc.Bacc("TRN2", target_bir_lowering=False, debug=False,
                   enable_asserts=False, num_devices=8)
    x = nc.dram_tensor("x", [S, E], F32, kind="ExternalInput").ap()
    w_qkv = nc.dram_tensor("w_qkv", [E, 1536], MM_QKV,
                           kind="ExternalInput").ap()
    w_proj = nc.dram_tensor("w_proj", [512, E], MM_PROJ,
                            kind="ExternalInput").ap()
    out = nc.dram_tensor("out", [S, E], F32, kind="ExternalOutput").ap()
    with tile.TileContext(nc) as tc:
        with ExitStack() as ctx:
            emit_kernel(ctx, tc, out, x, w_qkv, w_proj)
    nc.compile()
    _CACHE["nc"] = nc
    return nc


def _round_fp32r(a):
    """Round-to-nearest-even fp32 -> fp32r (11-bit mantissa), as numpy f32."""
    bits = np.ascontiguousarray(a, dtype=np.float32).view(np.uint32)
    keep = np.uint32(0xFFFFF000)
    half = np.uint32(0x800)
    lsb = (bits >> np.uint32(12)) & np.uint32(1)
    rounded = (bits + (half - np.uint32(1) + lsb)) & keep
    return rounded.view(np.float32)


def make_in_maps(x, w_attn, w_proj):
    x = np.asarray(x, dtype=np.float32)
    w_attn = np.asarray(w_attn, dtype=np.float32)
    w_proj = np.asarray(w_proj, dtype=np.float32)
    in_maps = []
    for c in range(8):
        b, hg = divmod(c, 2)
        lo, hi = hg * 512, (hg + 1) * 512
        wq = w_attn[:, lo:hi]
        wk = w_attn[:, 1024 + lo:1024 + hi]
        wv = w_attn[:, 2048 + lo:2048 + hi]
        wqkv = np.ascontiguousarray(np.concatenate([wq, wk, wv], axis=1))
        wp = np.ascontiguousarray(w_proj[lo:hi, :])
        if MM_QKV == F32R:
            wqkv = _round_fp32r(wqkv)
        if MM_PROJ == F32R:
            wp = _round_fp32r(wp)
        in_maps.append({
            "x": np.ascontiguousarray(x[b]),
            "w_qkv": wqkv,
            "w_proj": wp,
        })
    return in_maps


def gather(results):
    parts = [results[c]["out"] for c in range(8)]
    return np.stack([parts[2 * b] + parts[2 * b + 1] for b in range(4)]).astype(
        np.float32)


def kernel(x, w_attn, w_proj):
    nc = build_nc()
    res = run_bass_kernel_spmd(nc, make_in_maps(x, w_attn, w_proj),
                               core_ids=list(range(8)))
    return gather(res.results)


# revision 37
# speedup vs baseline: 1.0135x; 1.0135x over previous
"""Causal self-attention Bass/Tile kernel for 8 Trainium2 NeuronCores.

Problem (hardcoded): x (4, 2048, 1024) f32, w_attn (1024, 3072), w_proj
(1024, 1024).  H=16 heads, D=64.  Output: (4, 2048, 1024) f32.

Sharding: core c handles batch b = c // 2 and head-group hg = c % 2
(8 heads each).  Data parallel on B, tensor parallel on heads: each core
gets the w_attn columns for its heads (q|k|v, each 512 cols) and the
w_proj rows for its heads (512 rows).  Per-core output is a partial sum
over head groups; the host adds the two partials per batch.

Per-core kernel structure (strips of 512 queries):
  phase 1: PE-transpose x strip -> x^T (exact f32); matmuls produce
           Q^T/K^T ([d, tok], head pairs stacked on partitions) and V
           ([tok, d], 8 heads side by side).
  phase 2: per head-pair, per key-tile t: scores^T = K^T.T @ Q^T
           (row-packed pair), exp on ACT (scale=1/8 folded in), causal
           mask multiply on diagonal tiles (DVE), then col-packed
           matmuls accumulate exp@V -> y^T and ones@exp -> row sums.
           Softmax normalization = DVE reciprocal of sums + multiply.
  phase 3: out = y^T.T @ w_proj, accumulated over the 4 local f-chunks
           (emitted one strip late so the PE has transpose work while
           DVE normalizes).

Matmul dtype is configurable per phase: float32 (exact, 4 cyc/row) or
float32r (fp32 with 11-bit mantissa, 1 cyc/row).  float32r operands must
be *produced* rounded: on-chip producers (DVE copies, ACT exp) write
f32r-typed tiles, and weights are pre-rounded on the host (the DRAM
tensors are declared f32r).

No softmax max-subtraction: scores for these inputs are ~N(0,1)
(measured |s| <= 8.4), exp is fp32-safe.

PSUM static budget (8 banks): ph1 shared tag x4 (transpose/qkv/proj),
ps x2 (scores), py x1 (exp@V accum), psm x1 (softmax sums accum).
"""

import os
from contextlib import ExitStack

import numpy as np

import concourse.bass as bass
import concourse.bacc as bacc
import concourse.mybir as mybir
import concourse.tile as tile
from concourse.bass_utils import run_bass_kernel_spmd
from concourse.masks import make_identity

F32 = mybir.dt.float32
F32R = mybir.dt.float32r
EXP = mybir.ActivationFunctionType.Exp

S = 2048          # sequence length
E = 1024          # embedding
D = 64            # head dim
HL = 8            # heads per core
NP = 4            # head pairs per core
EC = 8            # E / 128 chunks
NSTRIP = 4        # query strips of 512
TPS = 4           # 128-token tiles per strip
NT = 16           # 128-key tiles total

_DT = {"f32": F32, "f32r": F32R}
MM_QKV = _DT[os.environ.get("MM_QKV", "f32r")]
MM_ATT = _DT[os.environ.get("MM_ATT", "f32r")]
MM_PROJ = _DT[os.environ.get("MM_PROJ", "f32r")]


def emit_kernel(ctx, tc, out, x, w_qkv, w_proj):
    nc = tc.nc

    const = ctx.enter_context(tc.tile_pool(name="const", bufs=1))
    wpool = ctx.enter_context(tc.tile_pool(name="weights", bufs=1))
    kv = ctx.enter_context(tc.tile_pool(name="kv", bufs=1))
    work = ctx.enter_context(tc.tile_pool(name="work", bufs=1))
    psum = ctx.enter_context(tc.tile_pool(name="psum", bufs=1, space="PSUM"))

    # ---- constants ----
    ident = const.tile([128, 128], F32, name="ident")
    make_identity(nc, ident)
    # ones column source for the V||1 augmented tiles (f32; rounded on copy)
    ones_row8 = const.tile([128, 8], F32, name="ones_row8")
    nc.gpsimd.memset(ones_row8[:], 1.0)
    # DRAM bounce rows for the softmax-sums broadcast (2 per pair-strip)
    rbounce = nc.dram_tensor("rbounce", [2 * NP * NSTRIP, 512], F32).ap()
    # causal mask template: T[i, jj] = 1 iff jj >= i + 384.
    # mask variant p (key tile 4s+p vs strip s) = T[:, 384-128p : 896-128p]
    mask_t = const.tile([128, 896], F32, name="mask_t")
    nc.gpsimd.memset(mask_t[:], 1.0)
    nc.gpsimd.affine_select(
        out=mask_t[:],
        in_=mask_t[:],
        compare_op=mybir.AluOpType.is_ge,
        fill=0.0,
        base=-384,
        channel_multiplier=-1,
        pattern=[[1, 896]],
    )

    # ---- resident weights (DRAM already in matmul dtype, host-rounded) ----
    wqk = []
    for e in range(EC):
        t = wpool.tile([128, 1024], MM_QKV, name=f"wqk{e}", tag=f"wqk{e}")
        nc.sync.dma_start(out=t[:], in_=w_qkv[e * 128:(e + 1) * 128, 0:1024])
        wqk.append(t)
    wv = []
    for e in range(EC):
        t = wpool.tile([128, 512], MM_QKV, name=f"wv{e}", tag=f"wv{e}")
        nc.sync.dma_start(out=t[:], in_=w_qkv[e * 128:(e + 1) * 128, 1024:1536])
        wv.append(t)
    wpj = []
    for f in range(NP):
        t = wpool.tile([128, 1024], MM_PROJ, name=f"wpj{f}", tag=f"wpj{f}")
        nc.sync.dma_start(out=t[:], in_=w_proj[f * 128:(f + 1) * 128, :])
        wpj.append(t)

    # ---- persistent K^T (pair-stacked) and V||ones (8 heads x 65) ----
    kT = [kv.tile([128, S], MM_ATT, name=f"kT{p}", tag=f"kT{p}")
          for p in range(NP)]
    vaug = [kv.tile([128, 520], MM_ATT, name=f"vaug_{t}", tag=f"vaug_{t}")
            for t in range(NT)]

    state = {}

    def transpose_chunk(s, tt, half):
        """Load + PE-transpose half an x tile of strip s into x^T."""
        if ("xT", s) not in state:
            state[("xT", s)] = [
                work.tile([128, 512], MM_QKV, name=f"xT{e}_{s}", tag=f"xT{e}")
                for e in range(EC)]
        xT = state[("xT", s)]
        xin = work.tile([128, 512], F32, name=f"xin_{s}_{tt}_{half}",
                        tag="xin", bufs=2)
        r0 = (s * TPS + tt) * 128
        nc.sync.dma_start(
            out=xin[:], in_=x[r0:r0 + 128, half * 512:(half + 1) * 512])
        for e4 in range(4):
            e = half * 4 + e4
            pt = psum.tile([128, 128], F32, name=f"pt_{s}_{tt}_{e}",
                           tag="ph1", bufs=3)
            nc.tensor.transpose(pt[:], xin[:, e4 * 128:(e4 + 1) * 128],
                                ident[:])
            nc.vector.tensor_copy(xT[e][:, tt * 128:(tt + 1) * 128], pt[:])

    def qk_chunk(s, p, which, half):
        """Half of the Q^T (or K^T) accumulation for pair p of strip s."""
        xT = state[("xT", s)]
        if ("qT", s) not in state:
            state[("qT", s)] = [
                work.tile([128, 512], MM_ATT, name=f"qT{p}_{s}",
                          tag=f"qT{p}", bufs=2)
                for p in range(NP)]
        qT = state[("qT", s)]
        co = (0 if which == "q" else 512) + p * 128
        if half == 0:
            pqk = psum.tile([128, 512], F32, name=f"p{which}_{s}_{p}",
                            tag="ph1", bufs=3)
            state[("pqk", s, p, which)] = pqk
        else:
            pqk = state.pop(("pqk", s, p, which))
        for e in range(4 * half, 4 * half + 4):
            nc.tensor.matmul(pqk[:], wqk[e][:, co:co + 128], xT[e][:],
                             start=(e == 0), stop=(e == EC - 1))
        if half == 1:
            if which == "q":
                nc.vector.tensor_copy(qT[p][:], pqk[:])
            else:
                nc.vector.tensor_copy(kT[p][:, s * 512:(s + 1) * 512], pqk[:])

    def v_chunk(s, tt, half):
        """Half of the V||ones accumulation for x tile tt of strip s."""
        xT = state[("xT", s)]
        if half == 0:
            pv = psum.tile([128, 512], F32, name=f"pv_{s}_{tt}", tag="ph1",
                           bufs=3)
            state[("pv", s, tt)] = pv
        else:
            pv = state.pop(("pv", s, tt))
        for e in range(4 * half, 4 * half + 4):
            nc.tensor.matmul(pv[:], xT[e][:, tt * 128:(tt + 1) * 128],
                             wv[e][:], start=(e == 0), stop=(e == EC - 1))
        if half == 1:
            # scatter V into the augmented [head*65 .. head*65+64] slots and
            # fill the ones columns, both as single strided copies
            va = vaug[s * TPS + tt]
            va3 = va.rearrange("p (h c) -> p h c", c=65)
            nc.vector.tensor_copy(va3[:, :, 0:64],
                                  pv[:].rearrange("p (h c) -> p h c", c=64))
            nc.vector.tensor_copy(va3[:, :, 64:65],
                                  ones_row8[:].rearrange("p (h c) -> p h c", c=1))

    def phase1_units(s):
        """Phase-1 work for strip s as fine-grained filler units (each a
        couple of us of PE work) for interleaving into the attention loop."""
        us = []
        for tt in range(TPS):
            for half in range(2):
                us.append(lambda s=s, tt=tt, h=half: transpose_chunk(s, tt, h))
        for p in range(NP):
            for which in ("q", "k"):
                for half in range(2):
                    us.append(lambda s=s, p=p, w=which, h=half:
                              qk_chunk(s, p, w, h))
        for tt in range(TPS):
            for half in range(2):
                us.append(lambda s=s, tt=tt, h=half: v_chunk(s, tt, h))
        return us

    def norm_units(s):
        """Deferred softmax normalization (one unit per pair of strip s)."""
        def norm(p):
            yu, recb = state.pop(("norm", s, p))
            yT = state[("yT", s)]
            nc.vector.reciprocal(recb[:], recb[:])
            nc.vector.tensor_mul(yT[p][:], yu[:], recb[:])
        return [lambda p=p: norm(p) for p in range(NP)]

    def p3_units(s):
        """Projection for strip s as units (one per output tile)."""
        def proj(tt, eo):
            yT = state[("yT", s)]
            po = psum.tile([128, 512], F32, name=f"po_{s}_{tt}_{eo}",
                           tag="ph1", bufs=3)
            for p in range(NP):
                nc.tensor.matmul(
                    po[:], yT[p][:, tt * 128:(tt + 1) * 128],
                    wpj[p][:, eo * 512:(eo + 1) * 512],
                    start=(p == 0), stop=(p == NP - 1))
            osb = work.tile([128, 512], F32, name=f"osb_{s}_{tt}_{eo}",
                            tag="osb", bufs=2)
            nc.vector.tensor_copy(osb[:], po[:])
            r0 = (s * TPS + tt) * 128
            nc.sync.dma_start(
                out=out[r0:r0 + 128, eo * 512:(eo + 1) * 512], in_=osb[:])
        return [lambda tt=tt, eo=eo: proj(tt, eo)
                for tt in range(TPS) for eo in range(2)]

    def phase2(s, units):
        """Attention for strip s.  `units` are independent emission closures
        drip-fed into the t-loop (roughly evenly across all pairs) so the PE
        always has fill work while ACT paces the exp stream."""
        qT = state[("qT", s)]
        state[("yT", s)] = [
            work.tile([128, 512], MM_PROJ, name=f"yT{p}_{s}", tag=f"yT{p}")
            for p in range(NP)]
        ntile = 4 * s + 4
        units = list(units)
        nslots = NP * ntile
        rate = len(units) / nslots
        pulled = 0
        slot = 0

        def pull():
            nonlocal pulled, slot
            slot += 1
            while pulled < len(units) and pulled < rate * slot:
                units[pulled]()
                pulled += 1

        for p in range(NP):
            py_a = psum.tile([65, 512], F32, name=f"pya_{s}_{p}", tag="py",
                             bufs=2)
            py_b = psum.tile([65, 512], F32, name=f"pyb_{s}_{p}", tag="py",
                             bufs=2)

            def scores_exp(t):
                # diagonal tiles: columns below 128*dshift are fully masked,
                # so compute only [c0:512] (c0 capped at 256 to keep the
                # f32r matmul in its fast >=256-free-dim regime)
                dshift = t - 4 * s
                c0 = 0 if dshift < 0 else min(128 * dshift, 256)
                ksl = kT[p][:, t * 128:(t + 1) * 128]
                ps_a = psum.tile([128, 512], F32, name=f"psa_{s}_{p}_{t}",
                                 tag="ps", bufs=3)
                ps_b = psum.tile([128, 512], F32, name=f"psb_{s}_{p}_{t}",
                                 tag="ps", bufs=3)
                nc.tensor.matmul(ps_a[:, c0:], ksl[0:64, :], qT[p][0:64, c0:],
                                 start=True, stop=True)
                nc.tensor.matmul(ps_b[:, c0:], ksl[64:128, :],
                                 qT[p][64:128, c0:],
                                 start=True, stop=True,
                                 tile_position=(64, 0))
                es_a = work.tile([128, 512], MM_ATT, name=f"esa_{s}_{p}_{t}",
                                 tag="es", bufs=4)
                es_b = work.tile([128, 512], MM_ATT, name=f"esb_{s}_{p}_{t}",
                                 tag="es", bufs=4)
                nc.scalar.activation(es_a[:, c0:], ps_a[:, c0:], EXP,
                                     scale=0.125)
                nc.scalar.activation(es_b[:, c0:], ps_b[:, c0:], EXP,
                                     scale=0.125)
                if dshift >= 0:  # causal mask on the partially-valid span
                    if dshift == 3:
                        sl, base, w = slice(256, 512), -128, 256
                    else:
                        sl = slice(128 * dshift, 128 * dshift + 128)
                        base, w = 0, 128
                    for est in (es_a, es_b):
                        nc.gpsimd.affine_select(
                            out=est[:, sl], in_=est[:, sl],
                            compare_op=mybir.AluOpType.is_ge, fill=0.0,
                            base=base, channel_multiplier=-1,
                            pattern=[[1, w]])
                return es_a, es_b, c0

            def av_sums(t, es_a, es_b, c0):
                st = (t == 0)
                sp = (t == ntile - 1)
                vA = vaug[t][:, (2 * p) * 65:(2 * p) * 65 + 65]
                vB = vaug[t][:, (2 * p + 1) * 65:(2 * p + 1) * 65 + 65]
                nc.tensor.matmul(py_a[:, c0:], vA, es_a[:, c0:],
                                 start=st, stop=sp)
                nc.tensor.matmul(py_b[:, c0:], vB, es_b[:, c0:],
                                 start=st, stop=sp)

            # software pipeline: issue scores(t+1) before exp@V(t) so the
            # PE never waits on ACT's exp; drip filler units in per slot.
            prev = scores_exp(0)
            for t in range(1, ntile):
                cur = scores_exp(t)
                av_sums(t - 1, *prev)
                pull()
                prev = cur
            av_sums(ntile - 1, *prev)
            pull()
            del prev

            # pair tail: move unnormalized y^T and the sums rows off PSUM
            # immediately (frees the py banks), bounce the sums through DRAM
            # to broadcast them, and defer the reciprocal+multiply to a
            # norm unit that runs early in the NEXT strip (by which time the
            # DMA round-trip has long landed -> no DVE stall).
            ri = 2 * (s * NP + p)
            yu = work.tile([128, 512], F32, name=f"yu_{s}_{p}",
                           tag=f"yu{p}", bufs=1)
            nc.vector.tensor_copy(yu[0:64, :], py_a[0:64, :])
            nc.vector.tensor_copy(yu[64:128, :], py_b[0:64, :])
            srab = work.tile([1, 1024], F32, name=f"srab_{s}_{p}",
                             tag="srab", bufs=1)
            nc.vector.tensor_copy(srab[:, 0:512], py_a[64:65, :])
            nc.vector.tensor_copy(srab[:, 512:1024], py_b[64:65, :])
            nc.scalar.dma_start(
                out=rbounce[ri:ri + 2, :].rearrange("a b -> (a b)").unsqueeze(0),
                in_=srab[:])
            recb = work.tile([128, 512], F32, name=f"recb_{s}_{p}",
                             tag="recb", bufs=2)
            nc.scalar.dma_start(
                out=recb[0:64, :],
                in_=rbounce[ri:ri + 1, :].broadcast_to((64, 512)))
            nc.scalar.dma_start(
                out=recb[64:128, :],
                in_=rbounce[ri + 1:ri + 2, :].broadcast_to((64, 512)))
            state[("norm", s, p)] = (yu, recb)
        while pulled < len(units):
            units[pulled]()
            pulled += 1

    def whole_body():
        state.clear()
        for u in phase1_units(0):
            u()
        for s in range(NSTRIP):
            units = []
            if s >= 1:
                units.extend(norm_units(s - 1))
            if s + 1 < NSTRIP:
                units.extend(phase1_units(s + 1))
            if s >= 1:
                units.extend(p3_units(s - 1))
            phase2(s, units)
        for u in norm_units(NSTRIP - 1) + p3_units(NSTRIP - 1):
            u()

    repeat = int(os.environ.get("KREPEAT", "1"))
    if repeat > 1:
        # timing-only mode: run the whole computation `repeat` times
        # (idempotent) so marginal wall-clock per iteration = HW exec time
        with tc.For_i(0, repeat, 1):
            whole_body()
    else:
        whole_body()


_CACHE = {}


def build_nc():
    if "nc" in _CACHE:
        return _CACHE["nc"]
    nc = bacc.Bacc("TRN2", target_bir_lowering=False, debug=False,
                   enable_asserts=False, num_devices=8)
    x = nc.dram_tensor("x", [S, E], F32, kind="ExternalInput").ap()
    w_qkv = nc.dram_tensor("w_qkv", [E, 1536], MM_QKV,
                           kind="ExternalInput").ap()
    w_proj = nc.dram_tensor("w_proj", [512, E], MM_PROJ,
                            kind="ExternalInput").ap()
    out = nc.dram_tensor("out", [S, E], F32, kind="ExternalOutput").ap()
    with tile.TileContext(nc) as tc:
        with ExitStack() as ctx:
            emit_kernel(ctx, tc, out, x, w_qkv, w_proj)
    nc.compile()
    _CACHE["nc"] = nc
    return nc


def _round_fp32r(a):
    """Round-to-nearest-even fp32 -> fp32r (11-bit mantissa), as numpy f32."""
    bits = np.ascontiguousarray(a, dtype=np.float32).view(np.uint32)
    keep = np.uint32(0xFFFFF000)
    half = np.uint32(0x800)
    lsb = (bits >> np.uint32(12)) & np.uint32(1)
    rounded = (bits + (half - np.uint32(1) + lsb)) & keep
    return rounded.view(np.float32)


def make_in_maps(x, w_attn, w_proj):
    x = np.asarray(x, dtype=np.float32)
    w_attn = np.asarray(w_attn, dtype=np.float32)
    w_proj = np.asarray(w_proj, dtype=np.float32)
    in_maps = []
    for c in range(8):
        b, hg = divmod(c, 2)
        lo, hi = hg * 512, (hg + 1) * 512
        wq = w_attn[:, lo:hi]
        wk = w_attn[:, 1024 + lo:1024 + hi]
        wv = w_attn[:, 2048 + lo:2048 + hi]
        wqkv = np.ascontiguousarray(np.concatenate([wq, wk, wv], axis=1))
        wp = np.ascontiguousarray(w_proj[lo:hi, :])
        if MM_QKV == F32R:
            wqkv = _round_fp32r(wqkv)
        if MM_PROJ == F32R:
            wp = _round_fp32r(wp)
        in_maps.append({
            "x": np.ascontiguousarray(x[b]),
            "w_qkv": wqkv,
            "w_proj": wp,
        })
    return in_maps


def gather(results):
    parts = [results[c]["out"] for c in range(8)]
    return np.stack([parts[2 * b] + parts[2 * b + 1] for b in range(4)]).astype(
        np.float32)


def kernel(x, w_attn, w_proj):
    nc = build_nc()
    res = run_bass_kernel_spmd(nc, make_in_maps(x, w_attn, w_proj),
                               core_ids=list(range(8)))
    return gather(res.results)


# revision 38
# speedup vs baseline: 239.4917x; 236.2931x over previous
"""Causal self-attention Bass/Tile kernel for 8 Trainium2 NeuronCores.

Problem (hardcoded): x (4, 2048, 1024) f32, w_attn (1024, 3072), w_proj
(1024, 1024).  H=16 heads, D=64.  Output: (4, 2048, 1024) f32.

Sharding: core c handles batch b = c // 2 and head-group hg = c % 2
(8 heads each).  Data parallel on B, tensor parallel on heads: each core
gets the w_attn columns for its heads (q|k|v, each 512 cols) and the
w_proj rows for its heads (512 rows).  Per-core output is a partial sum
over head groups; the host adds the two partials per batch.

Per-core kernel structure (strips of 512 queries):
  phase 1: PE-transpose x strip -> x^T (exact f32); matmuls produce
           Q^T/K^T ([d, tok], head pairs stacked on partitions) and V
           ([tok, d], 8 heads side by side).
  phase 2: per head-pair, per key-tile t: scores^T = K^T.T @ Q^T
           (row-packed pair), exp on ACT (scale=1/8 folded in), causal
           mask multiply on diagonal tiles (DVE), then col-packed
           matmuls accumulate exp@V -> y^T and ones@exp -> row sums.
           Softmax normalization = DVE reciprocal of sums + multiply.
  phase 3: out = y^T.T @ w_proj, accumulated over the 4 local f-chunks
           (emitted one strip late so the PE has transpose work while
           DVE normalizes).

Matmul dtype is configurable per phase: float32 (exact, 4 cyc/row) or
float32r (fp32 with 11-bit mantissa, 1 cyc/row).  float32r operands must
be *produced* rounded: on-chip producers (DVE copies, ACT exp) write
f32r-typed tiles, and weights are pre-rounded on the host (the DRAM
tensors are declared f32r).

No softmax max-subtraction: scores for these inputs are ~N(0,1)
(measured |s| <= 8.4), exp is fp32-safe.

PSUM static budget (8 banks): ph1 shared tag x4 (transpose/qkv/proj),
ps x2 (scores), py x1 (exp@V accum), psm x1 (softmax sums accum).
"""

import os
from contextlib import ExitStack

import numpy as np

import concourse.bass as bass
import concourse.bacc as bacc
import concourse.mybir as mybir
import concourse.tile as tile
from concourse.bass_utils import run_bass_kernel_spmd
from concourse.masks import make_identity

F32 = mybir.dt.float32
F32R = mybir.dt.float32r
EXP = mybir.ActivationFunctionType.Exp

S = 2048          # sequence length
E = 1024          # embedding
D = 64            # head dim
HL = 8            # heads per core
NP = 4            # head pairs per core
EC = 8            # E / 128 chunks
NSTRIP = 4        # query strips of 512
TPS = 4           # 128-token tiles per strip
NT = 16           # 128-key tiles total

_DT = {"f32": F32, "f32r": F32R}
MM_QKV = _DT[os.environ.get("MM_QKV", "f32r")]
MM_ATT = _DT[os.environ.get("MM_ATT", "f32r")]
MM_PROJ = _DT[os.environ.get("MM_PROJ", "f32r")]


def emit_kernel(ctx, tc, out, x, w_qkv, w_proj):
    nc = tc.nc

    const = ctx.enter_context(tc.tile_pool(name="const", bufs=1))
    wpool = ctx.enter_context(tc.tile_pool(name="weights", bufs=1))
    kv = ctx.enter_context(tc.tile_pool(name="kv", bufs=1))
    work = ctx.enter_context(tc.tile_pool(name="work", bufs=1))
    psum = ctx.enter_context(tc.tile_pool(name="psum", bufs=1, space="PSUM"))

    # ---- constants ----
    ident = const.tile([128, 128], F32, name="ident")
    make_identity(nc, ident)
    # ones column source for the V||1 augmented tiles (f32; rounded on copy)
    ones_row8 = const.tile([128, 8], F32, name="ones_row8")
    nc.gpsimd.memset(ones_row8[:], 1.0)
    # DRAM bounce rows for the softmax-sums broadcast (2 per pair-strip)
    rbounce = nc.dram_tensor("rbounce", [2 * NP * NSTRIP, 512], F32).ap()

    # ---- resident weights (DRAM already in matmul dtype, host-rounded) ----
    wqk = []
    for e in range(EC):
        t = wpool.tile([128, 1024], MM_QKV, name=f"wqk{e}", tag=f"wqk{e}")
        nc.sync.dma_start(out=t[:], in_=w_qkv[e * 128:(e + 1) * 128, 0:1024])
        wqk.append(t)
    wv = []
    for e in range(EC):
        t = wpool.tile([128, 512], MM_QKV, name=f"wv{e}", tag=f"wv{e}")
        nc.sync.dma_start(out=t[:], in_=w_qkv[e * 128:(e + 1) * 128, 1024:1536])
        wv.append(t)
    wpj = []
    for f in range(NP):
        t = wpool.tile([128, 1024], MM_PROJ, name=f"wpj{f}", tag=f"wpj{f}")
        nc.sync.dma_start(out=t[:], in_=w_proj[f * 128:(f + 1) * 128, :])
        wpj.append(t)

    # ---- persistent K^T (pair-stacked) and V||ones (8 heads x 65) ----
    kT = [kv.tile([128, S], MM_ATT, name=f"kT{p}", tag=f"kT{p}")
          for p in range(NP)]
    vaug = [kv.tile([128, 520], MM_ATT, name=f"vaug_{t}", tag=f"vaug_{t}")
            for t in range(NT)]

    state = {}

    def transpose_chunk(s, tt, half):
        """Load + PE-transpose half an x tile of strip s into x^T."""
        if ("xT", s) not in state:
            state[("xT", s)] = [
                work.tile([128, 512], MM_QKV, name=f"xT{e}_{s}", tag=f"xT{e}")
                for e in range(EC)]
        xT = state[("xT", s)]
        xin = work.tile([128, 512], F32, name=f"xin_{s}_{tt}_{half}",
                        tag="xin", bufs=2)
        r0 = (s * TPS + tt) * 128
        nc.scalar.dma_start(
            out=xin[:], in_=x[r0:r0 + 128, half * 512:(half + 1) * 512])
        for e4 in range(4):
            e = half * 4 + e4
            pt = psum.tile([128, 128], F32, name=f"pt_{s}_{tt}_{e}",
                           tag="ph1", bufs=3)
            nc.tensor.transpose(pt[:], xin[:, e4 * 128:(e4 + 1) * 128],
                                ident[:])
            nc.vector.tensor_copy(xT[e][:, tt * 128:(tt + 1) * 128], pt[:])

    def qk_chunk(s, p, which, half):
        """Half of the Q^T (or K^T) accumulation for pair p of strip s."""
        xT = state[("xT", s)]
        if ("qT", s) not in state:
            state[("qT", s)] = [
                work.tile([128, 512], MM_ATT, name=f"qT{p}_{s}",
                          tag=f"qT{p}", bufs=2)
                for p in range(NP)]
        qT = state[("qT", s)]
        co = (0 if which == "q" else 512) + p * 128
        if half == 0:
            pqk = psum.tile([128, 512], F32, name=f"p{which}_{s}_{p}",
                            tag="ph1", bufs=3)
            state[("pqk", s, p, which)] = pqk
        else:
            pqk = state.pop(("pqk", s, p, which))
        for e in range(4 * half, 4 * half + 4):
            nc.tensor.matmul(pqk[:], wqk[e][:, co:co + 128], xT[e][:],
                             start=(e == 0), stop=(e == EC - 1))
        if half == 1:
            if which == "q":
                nc.vector.tensor_copy(qT[p][:], pqk[:])
            else:
                nc.vector.tensor_copy(kT[p][:, s * 512:(s + 1) * 512], pqk[:])

    def v_chunk(s, tt, half):
        """Half of the V||ones accumulation for x tile tt of strip s."""
        xT = state[("xT", s)]
        if half == 0:
            pv = psum.tile([128, 512], F32, name=f"pv_{s}_{tt}", tag="ph1",
                           bufs=3)
            state[("pv", s, tt)] = pv
        else:
            pv = state.pop(("pv", s, tt))
        for e in range(4 * half, 4 * half + 4):
            nc.tensor.matmul(pv[:], xT[e][:, tt * 128:(tt + 1) * 128],
                             wv[e][:], start=(e == 0), stop=(e == EC - 1))
        if half == 1:
            # scatter V into the augmented [head*65 .. head*65+64] slots and
            # fill the ones columns, both as single strided copies
            va = vaug[s * TPS + tt]
            va3 = va.rearrange("p (h c) -> p h c", c=65)
            nc.vector.tensor_copy(va3[:, :, 0:64],
                                  pv[:].rearrange("p (h c) -> p h c", c=64))
            nc.vector.tensor_copy(va3[:, :, 64:65],
                                  ones_row8[:].rearrange("p (h c) -> p h c", c=1))

    def phase1_units(s):
        """Phase-1 work for strip s as fine-grained filler units (each a
        couple of us of PE work) for interleaving into the attention loop."""
        us = []
        for tt in range(TPS):
            for half in range(2):
                us.append(lambda s=s, tt=tt, h=half: transpose_chunk(s, tt, h))
        for p in range(NP):
            for which in ("q", "k"):
                for half in range(2):
                    us.append(lambda s=s, p=p, w=which, h=half:
                              qk_chunk(s, p, w, h))
        for tt in range(TPS):
            for half in range(2):
                us.append(lambda s=s, tt=tt, h=half: v_chunk(s, tt, h))
        return us

    def norm_units(s):
        """Deferred softmax normalization (one unit per pair of strip s)."""
        def norm(p):
            yu, recb = state.pop(("norm", s, p))
            yT = state[("yT", s)]
            nc.vector.reciprocal(recb[:], recb[:])
            nc.vector.tensor_mul(yT[p][:], yu[:], recb[:])
        return [lambda p=p: norm(p) for p in range(NP)]

    def p3_units(s):
        """Projection for strip s as units (one per output tile)."""
        def proj(tt, eo):
            yT = state[("yT", s)]
            po = psum.tile([128, 512], F32, name=f"po_{s}_{tt}_{eo}",
                           tag="ph1", bufs=3)
            for p in range(NP):
                nc.tensor.matmul(
                    po[:], yT[p][:, tt * 128:(tt + 1) * 128],
                    wpj[p][:, eo * 512:(eo + 1) * 512],
                    start=(p == 0), stop=(p == NP - 1))
            osb = work.tile([128, 512], F32, name=f"osb_{s}_{tt}_{eo}",
                            tag="osb", bufs=2)
            nc.vector.tensor_copy(osb[:], po[:])
            r0 = (s * TPS + tt) * 128
            nc.sync.dma_start(
                out=out[r0:r0 + 128, eo * 512:(eo + 1) * 512], in_=osb[:])
        return [lambda tt=tt, eo=eo: proj(tt, eo)
                for tt in range(TPS) for eo in range(2)]

    def phase2(s, units):
        """Attention for strip s.  `units` are independent emission closures
        drip-fed into the t-loop (roughly evenly across all pairs) so the PE
        always has fill work while ACT paces the exp stream."""
        qT = state[("qT", s)]
        state[("yT", s)] = [
            work.tile([128, 512], MM_PROJ, name=f"yT{p}_{s}", tag=f"yT{p}")
            for p in range(NP)]
        ntile = 4 * s + 4
        units = list(units)
        nslots = NP * ntile
        rate = len(units) / nslots
        pulled = 0
        slot = 0

        def pull():
            nonlocal pulled, slot
            slot += 1
            while pulled < len(units) and pulled < rate * slot:
                units[pulled]()
                pulled += 1

        for p in range(NP):
            py_a = psum.tile([65, 512], F32, name=f"pya_{s}_{p}", tag="py",
                             bufs=2)
            py_b = psum.tile([65, 512], F32, name=f"pyb_{s}_{p}", tag="py",
                             bufs=2)

            def scores_exp(t):
                # diagonal tiles: columns below 128*dshift are fully masked,
                # so compute only [c0:512] (c0 capped at 256 to keep the
                # f32r matmul in its fast >=256-free-dim regime)
                dshift = t - 4 * s
                c0 = 0 if dshift < 0 else min(128 * dshift, 256)
                ksl = kT[p][:, t * 128:(t + 1) * 128]
                ps_a = psum.tile([128, 512], F32, name=f"psa_{s}_{p}_{t}",
                                 tag="ps", bufs=3)
                ps_b = psum.tile([128, 512], F32, name=f"psb_{s}_{p}_{t}",
                                 tag="ps", bufs=3)
                nc.tensor.matmul(ps_a[:, c0:], ksl[0:64, :], qT[p][0:64, c0:],
                                 start=True, stop=True)
                nc.tensor.matmul(ps_b[:, c0:], ksl[64:128, :],
                                 qT[p][64:128, c0:],
                                 start=True, stop=True,
                                 tile_position=(64, 0))
                es_a = work.tile([128, 512], MM_ATT, name=f"esa_{s}_{p}_{t}",
                                 tag="es", bufs=6)
                es_b = work.tile([128, 512], MM_ATT, name=f"esb_{s}_{p}_{t}",
                                 tag="es", bufs=6)
                nc.scalar.activation(es_a[:, c0:], ps_a[:, c0:], EXP,
                                     scale=0.125)
                nc.scalar.activation(es_b[:, c0:], ps_b[:, c0:], EXP,
                                     scale=0.125)
                if dshift >= 0:  # causal mask on the partially-valid span
                    if dshift == 3:
                        sl, base, w = slice(256, 512), -128, 256
                    else:
                        sl = slice(128 * dshift, 128 * dshift + 128)
                        base, w = 0, 128
                    for est in (es_a, es_b):
                        nc.gpsimd.affine_select(
                            out=est[:, sl], in_=est[:, sl],
                            compare_op=mybir.AluOpType.is_ge, fill=0.0,
                            base=base, channel_multiplier=-1,
                            pattern=[[1, w]])
                return es_a, es_b, c0

            def av_sums(t, es_a, es_b, c0):
                st = (t == 0)
                sp = (t == ntile - 1)
                vA = vaug[t][:, (2 * p) * 65:(2 * p) * 65 + 65]
                vB = vaug[t][:, (2 * p + 1) * 65:(2 * p + 1) * 65 + 65]
                nc.tensor.matmul(py_a[:, c0:], vA, es_a[:, c0:],
                                 start=st, stop=sp)
                nc.tensor.matmul(py_b[:, c0:], vB, es_b[:, c0:],
                                 start=st, stop=sp)

            # software pipeline: issue scores(t+1) before exp@V(t) so the
            # PE never waits on ACT's exp; drip filler units in per slot.
            prev = scores_exp(0)
            for t in range(1, ntile):
                cur = scores_exp(t)
                av_sums(t - 1, *prev)
                pull()
                prev = cur
            av_sums(ntile - 1, *prev)
            pull()
            del prev

            # pair tail: move unnormalized y^T and the sums rows off PSUM
            # immediately (frees the py banks), bounce the sums through DRAM
            # to broadcast them, and defer the reciprocal+multiply to a
            # norm unit that runs early in the NEXT strip (by which time the
            # DMA round-trip has long landed -> no DVE stall).
            ri = 2 * (s * NP + p)
            yu = work.tile([128, 512], F32, name=f"yu_{s}_{p}",
                           tag=f"yu{p}", bufs=1)
            nc.vector.tensor_copy(yu[0:64, :], py_a[0:64, :])
            nc.vector.tensor_copy(yu[64:128, :], py_b[0:64, :])
            srab = work.tile([1, 1024], F32, name=f"srab_{s}_{p}",
                             tag="srab", bufs=1)
            nc.vector.tensor_copy(srab[:, 0:512], py_a[64:65, :])
            nc.vector.tensor_copy(srab[:, 512:1024], py_b[64:65, :])
            nc.scalar.dma_start(
                out=rbounce[ri:ri + 2, :].rearrange("a b -> (a b)").unsqueeze(0),
                in_=srab[:])
            recb = work.tile([128, 512], F32, name=f"recb_{s}_{p}",
                             tag="recb", bufs=2)
            nc.scalar.dma_start(
                out=recb[0:64, :],
                in_=rbounce[ri:ri + 1, :].broadcast_to((64, 512)))
            nc.scalar.dma_start(
                out=recb[64:128, :],
                in_=rbounce[ri + 1:ri + 2, :].broadcast_to((64, 512)))
            state[("norm", s, p)] = (yu, recb)
        while pulled < len(units):
            units[pulled]()
            pulled += 1

    def whole_body():
        state.clear()
        for u in phase1_units(0):
            u()
        for s in range(NSTRIP):
            units = []
            if s >= 1:
                units.extend(norm_units(s - 1))
            if s + 1 < NSTRIP:
                units.extend(phase1_units(s + 1))
            if s >= 1:
                units.extend(p3_units(s - 1))
            phase2(s, units)
        for u in norm_units(NSTRIP - 1) + p3_units(NSTRIP - 1):
            u()

    repeat = int(os.environ.get("KREPEAT", "1"))
    if repeat > 1:
        # timing-only mode: run the whole computation `repeat` times
        # (idempotent) so marginal wall-clock per iteration = HW exec time
        with tc.For_i(0, repeat, 1):
            whole_body()
    else:
        whole_body()


_CACHE = {}


def build_nc():
    if "nc" in _CACHE:
        return _CACHE["nc"]
    nc = bacc.Bacc("TRN2", target_bir_lowering=False, debug=False,
                   enable_asserts=False, num_devices=8)
    x = nc.dram_tensor("x", [S, E], F32, kind="ExternalInput").ap()
    w_qkv = nc.dram_tensor("w_qkv", [E, 1536], MM_QKV,
                           kind="ExternalInput").ap()
    w_proj = nc.dram_tensor("w_proj", [512, E], MM_PROJ,
                            kind="ExternalInput").ap()
    out = nc.dram_tensor("out", [S, E], F32, kind="ExternalOutput").ap()
    with tile.TileContext(nc) as tc:
        with ExitStack() as ctx:
            emit_kernel(ctx, tc, out, x, w_qkv, w_proj)
    nc.compile()
    _CACHE["nc"] = nc
    return nc


def _round_fp32r(a):
    """Round-to-nearest-even fp32 -> fp32r (11-bit mantissa), as numpy f32."""
    bits = np.ascontiguousarray(a, dtype=np.float32).view(np.uint32)
    keep = np.uint32(0xFFFFF000)
    half = np.uint32(0x800)
    lsb = (bits >> np.uint32(12)) & np.uint32(1)
    rounded = (bits + (half - np.uint32(1) + lsb)) & keep
    return rounded.view(np.float32)


def make_in_maps(x, w_attn, w_proj):
    x = np.asarray(x, dtype=np.float32)
    w_attn = np.asarray(w_attn, dtype=np.float32)
    w_proj = np.asarray(w_proj, dtype=np.float32)
    in_maps = []
    for c in range(8):
        b, hg = divmod(c, 2)
        lo, hi = hg * 512, (hg + 1) * 512
        wq = w_attn[:, lo:hi]
        wk = w_attn[:, 1024 + lo:1024 + hi]
        wv = w_attn[:, 2048 + lo:2048 + hi]
        wqkv = np.ascontiguousarray(np.concatenate([wq, wk, wv], axis=1))
        wp = np.ascontiguousarray(w_proj[lo:hi, :])
        if MM_QKV == F32R:
            wqkv = _round_fp32r(wqkv)
        if MM_PROJ == F32R:
            wp = _round_fp32r(wp)
        in_maps.append({
            "x": np.ascontiguousarray(x[b]),
            "w_qkv": wqkv,
            "w_proj": wp,
        })
    return in_maps


def gather(results):
    parts = [results[c]["out"] for c in range(8)]
    return np.stack([parts[2 * b] + parts[2 * b + 1] for b in range(4)]).astype(
        np.float32)


def kernel(x, w_attn, w_proj):
    nc = build_nc()
    res = run_bass_kernel_spmd(nc, make_in_maps(x, w_attn, w_proj),
                               core_ids=list(range(8)))
    return gather(res.results)
